# revision 37
# baseline (speedup 1.0000x reference)
"""Trainium2 Bass kernel for coverage (Bahdanau-style) attention.

Reference computation (B=32, S=2048, H=1024):
    enc_feature = encoder_outputs @ W_enc.T                    # [B,S,H]
    dec_feature = decoder_hidden @ W_dec.T + b_dec             # [B,1,H]
    cov_feature = coverage[..., None] * w_cov                  # [B,S,H]
    scores      = tanh(enc_feature + dec_feature + cov_feature)
    attn_scores = scores @ v                                   # [B,S]
    attn_dist   = softmax(attn_scores, axis=-1)[:, None, :]    # [B,1,S]

Sharding: data-parallel over batch B across 8 cores (4 batches/core).

Per-core device kernel — importance-weighted fp8 DoubleRow scheme:
  - Main matmul in fp8e4 DoubleRow (0.5 cyc/col covering 2 k-subtiles).
    Operands split hi/lo with error feedback, but the correction passes
    (el@Wh and eh@Wl) only run on the output channels that matter: the
    final attn error is sum_h v_h * tanh'(x_h) * dx_h, so channels are
    PERMUTED by |v| descending on the host and corrections restricted to
    the top NEL=NWL=3 of 8 chunks (~85% of the v^2 mass). Measured
    end-to-end rel err 1.35e-2 vs the fp32 reference (gate 2e-2,
    matches the numpy prediction within 3e-4). PE cost: 56 DR matmuls
    per 512-row block vs 96 for the full 3-pass scheme.
  - The coverage rank-1 term is FOLDED INTO e ON THE HOST: e' = e +
    cov[:,None]*u where u solves u @ (32*W^T) ~ 32*w_cov via SVD
    truncated at sigma >= 0.01*sigma_max (keeps |u|_inf ~ 0.7 so e'
    still quantizes cleanly to fp8; the dropped small-singular residual
    contributes ~1e-3 rel err). No cov DMA, no broadcast, no DVE fuse.
  - W pre-scaled by 32 on host so Wl stays out of fp8 subnormal
    underflow; tanh applies scale=1/32 to compensate.
  - dec_feature (+b_dec) computed on host, fused as tanh per-partition
    bias.
  - v-dot: tanh output tt (bf16) multiply-accumulated per h-chunk on DVE
    (scalar_tensor_tensor gets no DVE 2x modes, but at 8 x ~0.6us the
    chain fits under the PE cadence), summed across partitions with
    gpsimd.partition_all_reduce. The LAST block instead does the v-dot
    on the (tail-idle) PE as bf16 matmuls so the tail chain is short.
  - softmax: exp on ACT per block, deferred past the NEXT block's first
    tanh so tanh never queues behind it; the normalization (divide by
    the row sum) happens on the HOST after the gather, like
    dec_feature. One out-DMA per completed batch row keeps the 632ns
    HWDGE gen rare on the ACT sequencer; the last batch streams 3/4 of
    its row on the idle sync queue during the final block so only a
    [1,NF] DMA remains in the tail.
  - eh/el stream DMAs ride the sync queue 3 blocks deep (bufs=3);
    out-DMAs live on the scalar queue so a pending exp never blocks the
    stream (the cost model serializes all DMA, round-robining the two
    HWDGE queues — the lead-in queue split makes the alternation
    deliver tensors in first-need order).
  - Steady blocks interleave heavy (12 DR) and light (4 DR) chunks so
    chunk completion paces ~854ns and ACT (612ns/tanh) keeps up; PSUM
    (7 pm banks + 1 pv) never backs up into the PE.
  - PE warmup matmuls fill the initial DMA window (keeps the p-state
    clock ramp warm so real matmuls run at 2.4 GHz).

Engine budget per 512-row block (16 blocks/core): PE 56 DR = 5.98us;
ACT 8 tanh + exp ~ 5.7us; DVE v-dot ~ 4.6us; Pool all_reduce ~ 0.8us.
PE-bound with ACT close behind. TimelineSim: 112249 ns (baseline
178970).
"""

import os

os.environ.setdefault("JAX_PLATFORMS", "axon,cpu")

import ml_dtypes
import numpy as np

import concourse.bass as bass
import concourse.bass_isa as bass_isa
import concourse.mybir as mybir
import concourse.tile as tile
from concourse import bacc
from concourse.bass_utils import run_bass_kernel_spmd

B, S, H = 32, 2048, 1024
NCORES = 8
BC = B // NCORES          # batches per core
R = BC * S                # rows per core
P = 128
NF = 512                  # matmul moving free dim / row-block size
KC = H // P               # contraction subtiles of 128
MC = H // P               # h_out chunks
NRB = R // NF             # row blocks per core
RB_PER_B = S // NF        # row blocks per batch
ALPHA = 32.0              # host-side W scale (undone by tanh scale=1/32)
NEL = 3                   # top chunks getting the el@Wh correction
NWL = 3                   # top chunks getting the eh@Wl correction
FOLD_EPS = 0.01           # SVD cutoff for the coverage fold

F32 = mybir.dt.float32
F8 = mybir.dt.float8e4
BF16 = mybir.dt.bfloat16
E4NP = ml_dtypes.float8_e4m3
DR = mybir.MatmulPerfMode.DoubleRow

_CACHE = {}


def build():
    nc = bacc.Bacc(None, target_bir_lowering=False)

    eh_d = nc.dram_tensor("eh", [H, R], F8, kind="ExternalInput")
    el_d = nc.dram_tensor("el", [H, R], F8, kind="ExternalInput")
    # W hi/lo pre-rearranged on host to [p][(m, k, c)] so any m-chunk DMA is
    # fully contiguous per partition. wl only carries the top NWL chunks.
    wh_d = nc.dram_tensor("wh", [P, MC * KC * P], F8, kind="ExternalInput")
    wl_d = nc.dram_tensor("wl", [P, NWL * KC * P], F8, kind="ExternalInput")
    # packed small constants: v | dec  ([P, MC * (1 + BC)]) — one DMA
    cst_d = nc.dram_tensor("cst", [P, MC * (1 + BC)], F32, kind="ExternalInput")
    vb_d = nc.dram_tensor("vb", [P, MC], BF16, kind="ExternalInput")
    out_d = nc.dram_tensor("attn", [BC, S], F32, kind="ExternalOutput")

    with tile.TileContext(nc) as tc:
        with (
            tc.tile_pool(name="const", bufs=1) as const,
            tc.tile_pool(name="stream", bufs=3) as stream,
            tc.tile_pool(name="ttp", bufs=12) as ttp,
            tc.tile_pool(name="vtp", bufs=4) as vtp,
            tc.tile_pool(name="scp", bufs=3) as scp,
            tc.tile_pool(name="sm", bufs=2) as smp,
            tc.tile_pool(name="psm", bufs=7, space="PSUM") as psm,
            tc.tile_pool(name="psv", bufs=1, space="PSUM") as psv,
        ):
            wh_sb = const.tile([P, MC, KC, P], F8)
            wl_sb = const.tile([P, NWL, KC, P], F8)
            eh0 = stream.tile([P, KC, NF], F8, tag="eh")
            el0 = stream.tile([P, KC, NF], F8, tag="el")
            cst_sb = const.tile([P, MC * (1 + BC)], F32)
            vb_sb = const.tile([P, MC], BF16)
            wup = const.tile([P, MC], F8)
            v_sb = cst_sb[:, 0:MC]
            dec_sb = cst_sb[:, MC:].rearrange("p (m b) -> p m b", b=BC)

            # Warmup source must be initialized before the PE touches it.
            nc.vector.memset(wup[:], 0.0)

            def dma_w(dram, sb, lo, hi, q=None):
                (q or nc.scalar).dma_start(
                    sb[:, lo:hi, :, :],
                    dram.ap()[:, lo * KC * P : hi * KC * P].rearrange(
                        "p (m k c) -> p m k c", k=KC, c=P
                    ),
                )

            def dma_e(tile_, dram, r0, q):
                q.dma_start(
                    tile_[:],
                    dram.ap()[:, r0 : r0 + NF].rearrange("(k p) r -> p k r", p=P),
                )

            # The cost model executes ALL DMA transfers serially (single
            # DMA_ENGINES resource) with the two HWDGE queues strictly
            # round-robined, so split the lead-in DMAs across the two queues
            # so the ALTERNATION yields the global first-need order:
            #   eh0, wh01, cst, el0, wl, wh23, wh45, wh67, eh1, el1, eh2, el2
            eh1 = stream.tile([P, KC, NF], F8, tag="eh")
            el1 = stream.tile([P, KC, NF], F8, tag="el")
            eh2 = stream.tile([P, KC, NF], F8, tag="eh")
            el2 = stream.tile([P, KC, NF], F8, tag="el")
            dma_e(eh0, eh_d, 0, nc.sync)                    # s1
            dma_w(wh_d, wh_sb, 0, 2)                        # c1
            nc.sync.dma_start(cst_sb[:], cst_d.ap())        # s2
            dma_e(el0, el_d, 0, nc.scalar)                  # c2
            nc.sync.dma_start(                              # s3
                wl_sb[:],
                wl_d.ap().rearrange("p (m k c) -> p m k c", k=KC, c=P),
            )
            dma_w(wh_d, wh_sb, 2, 4)                        # c3
            dma_w(wh_d, wh_sb, 4, 6, q=nc.sync)             # s4
            dma_w(wh_d, wh_sb, 6, 8)                        # c4
            dma_e(eh1, eh_d, NF, nc.sync)                   # s5
            dma_e(el1, el_d, NF, nc.scalar)                 # c5
            dma_e(eh2, eh_d, 2 * NF, nc.sync)               # s6
            dma_e(el2, el_d, 2 * NF, nc.scalar)             # c6
            nc.scalar.dma_start(vb_sb[:], vb_d.ap())

            # PE warmup: tiny matmuls fill the initial DMA window so the PE
            # p-state clock is fully ramped (and never resets) when the real
            # matmul stream begins (~5.05us when eh0+wh01 have landed).
            wpsum = psm.tile([P, NF], F32, tag="pm")
            for _ in range(710):
                nc.tensor.matmul(
                    wpsum[0:MC, 0:MC], wup[:], wup[:], start=True, stop=True
                )

            ex = None
            pending_exp = None
            for rb in range(NRB):
                b = rb // RB_PER_B
                i = rb % RB_PER_B
                so = i * NF
                r0 = rb * NF

                if rb == 0:
                    eh, el = eh0, el0
                elif rb == 1:
                    eh, el = eh1, el1
                elif rb == 2:
                    eh, el = eh2, el2
                else:
                    eh = stream.tile([P, KC, NF], F8, tag="eh")
                    dma_e(eh, eh_d, r0, nc.sync)
                    el = stream.tile([P, KC, NF], F8, tag="el")
                    dma_e(el, el_d, r0, nc.sync)

                last = rb == NRB - 1

                if i == 0:
                    ex = smp.tile([1, S], F32, tag="ex")
                # On the last block the v-dot runs on the (tail-idle) PE as a
                # bf16 matmul instead of the DVE chain so the tail is short.
                vt = None if last else vtp.tile([P, NF], F32, tag="vt", name="vt")
                pv = psv.tile([1, NF], F32, tag="pv", name="pv") if last else None
                tts = [None] * MC

                # Phase schedule per block: each entry is (m, phase) with
                # phase 'h' = hi pass (group stays open), 'hC' = hi pass and
                # close, 'c' = corrections and close, 'f' = full chunk.
                # Steady blocks interleave heavy (3-pass) with light (1-pass)
                # chunks so chunk completion paces ~854ns — ACT (612ns/tanh)
                # keeps up and PSUM never backs up into the PE. Block 0
                # front-loads four hi passes so the PE has work while el0/wl
                # are still in flight on the serial DMA resource.
                if rb == 0:
                    sched = [(m, "f") for m in range(MC)]
                elif rb == NRB - 1:
                    # last block: light chunks first so the tanh chain (the
                    # pipeline drain) starts ~0.85us earlier
                    sched = [(m, "f") for m in [3, 4, 5, 6, 7, 0, 1, 2]]
                else:
                    sched = [(m, "f") for m in [0, 4, 1, 5, 2, 6, 3, 7]]

                pms = {}
                closed = []         # chunks in close order

                def mm_quad(pm, w_ap, mov, start, stop):
                    for k2 in range(KC // 2):
                        ks = slice(2 * k2, 2 * k2 + 2)
                        nc.tensor.matmul(
                            pm[:],
                            w_ap[:, ks, :],
                            mov[:, ks, :],
                            start=(start and k2 == 0),
                            stop=(stop and k2 == KC // 2 - 1),
                            perf_mode=DR,
                        )

                for m, phase in sched:
                    has_corr = m < NEL or m < NWL
                    if phase in ("h", "hC", "f"):
                        pm = psm.tile([P, NF], F32, tag="pm")
                        pms[m] = pm
                        mm_quad(
                            pm, wh_sb[:, m], eh,
                            start=True,
                            stop=(phase == "hC") or (phase == "f" and not has_corr),
                        )
                    if phase in ("c", "f") and has_corr:
                        pm = pms[m]
                        if m < NEL:
                            mm_quad(pm, wh_sb[:, m], el,
                                    start=False, stop=(m >= NWL))
                        if m < NWL:
                            mm_quad(pm, wl_sb[:, m], eh,
                                    start=False, stop=True)
                    if phase == "h":
                        continue

                    # chunk m is closed: tanh, deferred exp, v-dot
                    ci = len(closed)
                    closed.append(m)
                    tt = ttp.tile([P, NF], BF16, tag="tt")
                    nc.scalar.activation(
                        tt[:],
                        pms[m][:],
                        mybir.ActivationFunctionType.Tanh,
                        bias=dec_sb[:, m, b : b + 1],
                        scale=1.0 / ALPHA,
                    )
                    tts[m] = tt
                    if ci == 0 and pending_exp is not None:
                        # previous block's exp rides the ACT queue AFTER this
                        # block's first tanh, so tanh never waits behind it
                        p_ex, p_so, p_src, p_b = pending_exp
                        nc.scalar.activation(
                            p_ex[:, p_so : p_so + NF],
                            p_src,
                            mybir.ActivationFunctionType.Exp,
                        )
                        # one out-DMA per completed batch row (4 blocks):
                        # the ACT sequencer pays the 632ns HWDGE gen only
                        # once per batch, and on the ACT queue the exp
                        # dependency is already satisfied by queue order
                        if p_so == S - NF:
                            nc.scalar.dma_start(
                                out_d.ap()[p_b : p_b + 1, :], p_ex[:]
                            )
                        elif p_b == BC - 1 and p_so == S - 2 * NF:
                            # last batch: stream out the first 3 slices on
                            # the (idle) sync queue during the final block,
                            # leaving only [1,NF] for the tail DMA
                            nc.sync.dma_start(
                                out_d.ap()[p_b : p_b + 1, 0 : S - NF],
                                p_ex[:, 0 : S - NF],
                            )
                        pending_exp = None
                    if last:
                        # PE v-dot (bf16), deferred two chunks so it rides
                        # behind later chunks' matmuls instead of stalling
                        # the PE on the tanh chain
                        if ci >= 2:
                            pm2 = closed[ci - 2]
                            nc.tensor.matmul(
                                pv[:],
                                vb_sb[:, pm2 : pm2 + 1],
                                tts[pm2][:],
                                start=(ci == 2),
                                stop=False,
                            )
                        if ci == MC - 1:
                            for mj in (closed[ci - 1], m):
                                nc.tensor.matmul(
                                    pv[:],
                                    vb_sb[:, mj : mj + 1],
                                    tts[mj][:],
                                    start=False,
                                    stop=(mj == m),
                                )
                    elif ci == 0:
                        # v-dot accumulation on DVE: vt += tt * v[:,m]
                        nc.vector.tensor_scalar_mul(
                            vt[:], tt[:], v_sb[:, m : m + 1]
                        )
                    else:
                        nc.vector.scalar_tensor_tensor(
                            vt[:],
                            tt[:],
                            v_sb[:, m : m + 1],
                            vt[:],
                            mybir.AluOpType.mult,
                            mybir.AluOpType.add,
                        )

                if last:
                    # tail: emit exp + DMA immediately
                    nc.scalar.activation(
                        ex[:, so : so + NF],
                        pv[0:1, :],
                        mybir.ActivationFunctionType.Exp,
                    )
                    nc.sync.dma_start(
                        out_d.ap()[b : b + 1, S - NF :], ex[:, S - NF :]
                    )
                else:
                    sc = scp.tile([P, NF], F32, tag="sc", name="sc")
                    nc.gpsimd.partition_all_reduce(
                        sc[:], vt[:], P, bass_isa.ReduceOp.add
                    )
                    pending_exp = (ex, so, sc[0:1, :], b)

    nc.compile()
    return nc


def _get_nc():
    if "nc" not in _CACHE:
        _CACHE["nc"] = build()
    return _CACHE["nc"]


def prep_in_maps(decoder_hidden, encoder_outputs, coverage, W_enc, W_dec, b_dec, w_cov, v):
    decoder_hidden = np.asarray(decoder_hidden, dtype=np.float32)
    encoder_outputs = np.asarray(encoder_outputs, dtype=np.float32)
    coverage = np.asarray(coverage, dtype=np.float32)
    W_enc = np.asarray(W_enc, dtype=np.float32)
    W_dec = np.asarray(W_dec, dtype=np.float32)
    b_dec = np.asarray(b_dec, dtype=np.float32)
    w_cov = np.asarray(w_cov, dtype=np.float32)
    v = np.asarray(v, dtype=np.float32)

    # host-side tiny matmul: dec_feature [B, H]
    dec_feature = decoder_hidden[:, 0, :] @ W_dec.T + b_dec

    # Channel permutation by |v| descending: the attn error from dropped
    # correction passes scales with v_h^2, so corrections go to the top
    # chunks only.
    perm = np.argsort(-np.abs(v))
    vp = v[perm]
    Wp = W_enc[perm, :]
    wcovp = w_cov[perm]
    decp = dec_feature[:, perm]

    # W.T scaled by 32 (exact power of 2), split hi/lo into e4m3 with error
    # feedback. The x32 keeps Wl out of fp8 subnormal underflow.
    w32 = np.ascontiguousarray(Wp.T) * np.float32(ALPHA)      # [H(in), H(out)]
    wh8 = w32.astype(E4NP)
    wl8 = (w32 - wh8.astype(np.float32)).astype(E4NP)

    # Coverage fold: u s.t. u @ w32 ~ wcov*ALPHA via truncated SVD, so the
    # rank-1 cov term rides inside e and needs no device work at all.
    U, sv, Vt = np.linalg.svd(w32.astype(np.float64))
    keep = sv >= FOLD_EPS * sv[0]
    coef = Vt @ (wcovp.astype(np.float64) * ALPHA)
    u_fold = (U[:, keep] @ (coef[keep] / sv[keep])).astype(np.float32)

    def w_rearrange(w8, mc):
        # [H, mc*P] = [(k p), (m c)] -> [p, (m k c)] so per-m-chunk DMAs are
        # contiguous per partition
        return np.ascontiguousarray(
            w8.reshape(KC, P, mc, P).transpose(1, 2, 0, 3).reshape(P, mc * KC * P)
        )

    wh8 = w_rearrange(wh8, MC)
    wl8 = w_rearrange(wl8[:, : NWL * P], NWL)
    v_r = vp.reshape(MC, P).T                                 # [P, MC] f32
    vb_r = np.ascontiguousarray(v_r.astype(ml_dtypes.bfloat16))

    in_maps = []
    for c in range(NCORES):
        bs = slice(c * BC, (c + 1) * BC)
        e2 = encoder_outputs[bs] + coverage[bs][..., None] * u_fold
        encT = np.ascontiguousarray(e2.reshape(R, H).T)       # [H, R]
        eh8 = encT.astype(E4NP)
        el8 = (encT - eh8.astype(np.float32)).astype(E4NP)
        dec = decp[bs].T.reshape(MC, P, BC).transpose(1, 0, 2)  # [P, MC, BC]
        cst = np.ascontiguousarray(
            np.concatenate([v_r, dec.reshape(P, MC * BC)], axis=1).astype(
                np.float32
            )
        )
        in_maps.append(
            {
                "eh": eh8,
                "el": el8,
                "wh": wh8,
                "wl": wl8,
                "cst": cst,
                "vb": vb_r,
            }
        )
    return in_maps


def kernel(decoder_hidden, encoder_outputs, coverage, W_enc, W_dec, b_dec, w_cov, v):
    nc = _get_nc()
    in_maps = prep_in_maps(
        decoder_hidden, encoder_outputs, coverage, W_enc, W_dec, b_dec, w_cov, v
    )
    res = run_bass_kernel_spmd(nc, in_maps, core_ids=list(range(NCORES)))
    out = np.concatenate([r["attn"] for r in res.results], axis=0)  # [B, S] exp
    out = out / out.sum(axis=-1, keepdims=True)                     # normalize
    return out[:, None, :].astype(np.float32)                       # [B, 1, S]


# revision 38
# speedup vs baseline: 1.0001x; 1.0001x over previous
"""Trainium2 Bass kernel for coverage (Bahdanau-style) attention.

Reference computation (B=32, S=2048, H=1024):
    enc_feature = encoder_outputs @ W_enc.T                    # [B,S,H]
    dec_feature = decoder_hidden @ W_dec.T + b_dec             # [B,1,H]
    cov_feature = coverage[..., None] * w_cov                  # [B,S,H]
    scores      = tanh(enc_feature + dec_feature + cov_feature)
    attn_scores = scores @ v                                   # [B,S]
    attn_dist   = softmax(attn_scores, axis=-1)[:, None, :]    # [B,1,S]

Sharding: data-parallel over batch B across 8 cores (4 batches/core).

Per-core device kernel — importance-weighted fp8 DoubleRow scheme:
  - Main matmul in fp8e4 DoubleRow (0.5 cyc/col covering 2 k-subtiles).
    Operands split hi/lo with error feedback, but the correction passes
    (el@Wh and eh@Wl) only run on the output channels that matter: the
    final attn error is sum_h v_h * tanh'(x_h) * dx_h, so channels are
    PERMUTED by |v| descending on the host and corrections restricted to
    the top NEL=NWL=3 of 8 chunks (~85% of the v^2 mass). Measured
    end-to-end rel err 1.35e-2 vs the fp32 reference (gate 2e-2,
    matches the numpy prediction within 3e-4). PE cost: 56 DR matmuls
    per 512-row block vs 96 for the full 3-pass scheme.
  - The coverage rank-1 term is FOLDED INTO e ON THE HOST: e' = e +
    cov[:,None]*u where u solves u @ (32*W^T) ~ 32*w_cov via SVD
    truncated at sigma >= 0.01*sigma_max (keeps |u|_inf ~ 0.7 so e'
    still quantizes cleanly to fp8; the dropped small-singular residual
    contributes ~1e-3 rel err). No cov DMA, no broadcast, no DVE fuse.
  - W pre-scaled by 32 on host so Wl stays out of fp8 subnormal
    underflow; tanh applies scale=1/32 to compensate.
  - dec_feature (+b_dec) computed on host, fused as tanh per-partition
    bias.
  - v-dot: tanh output tt (bf16) multiply-accumulated per h-chunk on DVE
    (scalar_tensor_tensor gets no DVE 2x modes, but at 8 x ~0.6us the
    chain fits under the PE cadence), summed across partitions with
    gpsimd.partition_all_reduce. The LAST block instead does the v-dot
    on the (tail-idle) PE as bf16 matmuls so the tail chain is short.
  - softmax: exp on ACT per block, deferred past the NEXT block's first
    tanh so tanh never queues behind it; the normalization (divide by
    the row sum) happens on the HOST after the gather, like
    dec_feature. One out-DMA per completed batch row keeps the 632ns
    HWDGE gen rare on the ACT sequencer; the last batch streams 3/4 of
    its row on the idle sync queue during the final block so only a
    [1,NF] DMA remains in the tail.
  - eh/el stream DMAs ride the sync queue 3 blocks deep (bufs=3);
    out-DMAs live on the scalar queue so a pending exp never blocks the
    stream (the cost model serializes all DMA, round-robining the two
    HWDGE queues — the lead-in queue split makes the alternation
    deliver tensors in first-need order).
  - Steady blocks interleave heavy (12 DR) and light (4 DR) chunks so
    chunk completion paces ~854ns and ACT (612ns/tanh) keeps up; PSUM
    (7 pm banks + 1 pv) never backs up into the PE.
  - PE warmup matmuls fill the initial DMA window (keeps the p-state
    clock ramp warm so real matmuls run at 2.4 GHz).

Engine budget per 512-row block (16 blocks/core): PE 56 DR = 5.98us;
ACT 8 tanh + exp ~ 5.7us; DVE v-dot ~ 4.6us; Pool all_reduce ~ 0.8us.
PE-bound with ACT close behind. TimelineSim: 112249 ns (baseline
178970).
"""

import os

os.environ.setdefault("JAX_PLATFORMS", "axon,cpu")

import ml_dtypes
import numpy as np

import concourse.bass as bass
import concourse.bass_isa as bass_isa
import concourse.mybir as mybir
import concourse.tile as tile
from concourse import bacc
from concourse.bass_utils import run_bass_kernel_spmd

B, S, H = 32, 2048, 1024
NCORES = 8
BC = B // NCORES          # batches per core
R = BC * S                # rows per core
P = 128
NF = 512                  # matmul moving free dim / row-block size
KC = H // P               # contraction subtiles of 128
MC = H // P               # h_out chunks
NRB = R // NF             # row blocks per core
RB_PER_B = S // NF        # row blocks per batch
ALPHA = 32.0              # host-side W scale (undone by tanh scale=1/32)
NEL = 3                   # top chunks getting the el@Wh correction
EL_PAIRS = [4, 4, 2]      # k-pairs of el@Wh per corrected chunk
NWL = 3                   # top chunks getting the eh@Wl correction
FOLD_EPS = 0.01           # SVD cutoff for the coverage fold

F32 = mybir.dt.float32
F8 = mybir.dt.float8e4
BF16 = mybir.dt.bfloat16
E4NP = ml_dtypes.float8_e4m3
DR = mybir.MatmulPerfMode.DoubleRow

_CACHE = {}


def build():
    nc = bacc.Bacc(None, target_bir_lowering=False)

    eh_d = nc.dram_tensor("eh", [H, R], F8, kind="ExternalInput")
    el_d = nc.dram_tensor("el", [H, R], F8, kind="ExternalInput")
    # W hi/lo pre-rearranged on host to [p][(m, k, c)] so any m-chunk DMA is
    # fully contiguous per partition. wl only carries the top NWL chunks.
    wh_d = nc.dram_tensor("wh", [P, MC * KC * P], F8, kind="ExternalInput")
    wl_d = nc.dram_tensor("wl", [P, NWL * KC * P], F8, kind="ExternalInput")
    # packed small constants: v | dec  ([P, MC * (1 + BC)]) — one DMA
    cst_d = nc.dram_tensor("cst", [P, MC * (1 + BC)], F32, kind="ExternalInput")
    vb_d = nc.dram_tensor("vb", [P, MC], BF16, kind="ExternalInput")
    out_d = nc.dram_tensor("attn", [BC, S], F32, kind="ExternalOutput")

    with tile.TileContext(nc) as tc:
        with (
            tc.tile_pool(name="const", bufs=1) as const,
            tc.tile_pool(name="stream", bufs=3) as stream,
            tc.tile_pool(name="ttp", bufs=12) as ttp,
            tc.tile_pool(name="vtp", bufs=4) as vtp,
            tc.tile_pool(name="scp", bufs=3) as scp,
            tc.tile_pool(name="sm", bufs=2) as smp,
            tc.tile_pool(name="psm", bufs=7, space="PSUM") as psm,
            tc.tile_pool(name="psv", bufs=1, space="PSUM") as psv,
        ):
            wh_sb = const.tile([P, MC, KC, P], F8)
            wl_sb = const.tile([P, NWL, KC, P], F8)
            eh0 = stream.tile([P, KC, NF], F8, tag="eh")
            el0 = stream.tile([P, KC, NF], F8, tag="el")
            cst_sb = const.tile([P, MC * (1 + BC)], F32)
            vb_sb = const.tile([P, MC], BF16)
            wup = const.tile([P, MC], F8)
            v_sb = cst_sb[:, 0:MC]
            dec_sb = cst_sb[:, MC:].rearrange("p (m b) -> p m b", b=BC)

            # Warmup source must be initialized before the PE touches it.
            nc.vector.memset(wup[:], 0.0)

            def dma_w(dram, sb, lo, hi, q=None):
                (q or nc.scalar).dma_start(
                    sb[:, lo:hi, :, :],
                    dram.ap()[:, lo * KC * P : hi * KC * P].rearrange(
                        "p (m k c) -> p m k c", k=KC, c=P
                    ),
                )

            def dma_e(tile_, dram, r0, q):
                q.dma_start(
                    tile_[:],
                    dram.ap()[:, r0 : r0 + NF].rearrange("(k p) r -> p k r", p=P),
                )

            # The cost model executes ALL DMA transfers serially (single
            # DMA_ENGINES resource) with the two HWDGE queues strictly
            # round-robined, so split the lead-in DMAs across the two queues
            # so the ALTERNATION yields the global first-need order:
            #   eh0, wh01, cst, el0, wl, wh23, wh45, wh67, eh1, el1, eh2, el2
            eh1 = stream.tile([P, KC, NF], F8, tag="eh")
            el1 = stream.tile([P, KC, NF], F8, tag="el")
            eh2 = stream.tile([P, KC, NF], F8, tag="eh")
            el2 = stream.tile([P, KC, NF], F8, tag="el")
            dma_e(eh0, eh_d, 0, nc.sync)                    # s1
            dma_w(wh_d, wh_sb, 0, 2)                        # c1
            nc.sync.dma_start(cst_sb[:], cst_d.ap())        # s2
            dma_e(el0, el_d, 0, nc.scalar)                  # c2
            nc.sync.dma_start(                              # s3
                wl_sb[:],
                wl_d.ap().rearrange("p (m k c) -> p m k c", k=KC, c=P),
            )
            dma_w(wh_d, wh_sb, 2, 4)                        # c3
            dma_w(wh_d, wh_sb, 4, 6, q=nc.sync)             # s4
            dma_w(wh_d, wh_sb, 6, 8)                        # c4
            dma_e(eh1, eh_d, NF, nc.sync)                   # s5
            dma_e(el1, el_d, NF, nc.scalar)                 # c5
            dma_e(eh2, eh_d, 2 * NF, nc.sync)               # s6
            dma_e(el2, el_d, 2 * NF, nc.scalar)             # c6
            nc.scalar.dma_start(vb_sb[:], vb_d.ap())

            # PE warmup: tiny matmuls fill the initial DMA window so the PE
            # p-state clock is fully ramped (and never resets) when the real
            # matmul stream begins (~5.05us when eh0+wh01 have landed).
            wpsum = psm.tile([P, NF], F32, tag="pm")
            for _ in range(710):
                nc.tensor.matmul(
                    wpsum[0:MC, 0:MC], wup[:], wup[:], start=True, stop=True
                )

            ex = None
            pending_exp = None
            for rb in range(NRB):
                b = rb // RB_PER_B
                i = rb % RB_PER_B
                so = i * NF
                r0 = rb * NF

                if rb == 0:
                    eh, el = eh0, el0
                elif rb == 1:
                    eh, el = eh1, el1
                elif rb == 2:
                    eh, el = eh2, el2
                else:
                    eh = stream.tile([P, KC, NF], F8, tag="eh")
                    dma_e(eh, eh_d, r0, nc.sync)
                    el = stream.tile([P, KC, NF], F8, tag="el")
                    dma_e(el, el_d, r0, nc.sync)

                last = rb == NRB - 1

                if i == 0:
                    ex = smp.tile([1, S], F32, tag="ex")
                # On the last block the v-dot runs on the (tail-idle) PE as a
                # bf16 matmul instead of the DVE chain so the tail is short.
                vt = None if last else vtp.tile([P, NF], F32, tag="vt", name="vt")
                pv = psv.tile([1, NF], F32, tag="pv", name="pv") if last else None
                tts = [None] * MC

                # Phase schedule per block: each entry is (m, phase) with
                # phase 'h' = hi pass (group stays open), 'hC' = hi pass and
                # close, 'c' = corrections and close, 'f' = full chunk.
                # Steady blocks interleave heavy (3-pass) with light (1-pass)
                # chunks so chunk completion paces ~854ns — ACT (612ns/tanh)
                # keeps up and PSUM never backs up into the PE. Block 0
                # front-loads four hi passes so the PE has work while el0/wl
                # are still in flight on the serial DMA resource.
                if rb == 0:
                    sched = [(m, "f") for m in range(MC)]
                elif rb == NRB - 1:
                    # last block: light chunks first so the tanh chain (the
                    # pipeline drain) starts ~0.85us earlier
                    sched = [(m, "f") for m in [3, 4, 5, 6, 7, 0, 1, 2]]
                else:
                    sched = [(m, "f") for m in [0, 4, 1, 5, 2, 6, 3, 7]]

                pms = {}
                closed = []         # chunks in close order

                def mm_quad(pm, w_ap, mov, start, stop, pairs=KC // 2):
                    for k2 in range(pairs):
                        ks = slice(2 * k2, 2 * k2 + 2)
                        nc.tensor.matmul(
                            pm[:],
                            w_ap[:, ks, :],
                            mov[:, ks, :],
                            start=(start and k2 == 0),
                            stop=(stop and k2 == pairs - 1),
                            perf_mode=DR,
                        )

                for m, phase in sched:
                    has_corr = m < NEL or m < NWL
                    if phase in ("h", "hC", "f"):
                        pm = psm.tile([P, NF], F32, tag="pm")
                        pms[m] = pm
                        mm_quad(
                            pm, wh_sb[:, m], eh,
                            start=True,
                            stop=(phase == "hC") or (phase == "f" and not has_corr),
                        )
                    if phase in ("c", "f") and has_corr:
                        pm = pms[m]
                        if m < NEL:
                            # chunk 2's el correction covers only half the
                            # contraction (k-pairs 0-1): the dropped half
                            # adds sqrt(0.5)x of that chunk's el error,
                            # measured 1.60e-2 end to end (numpy == device)
                            mm_quad(pm, wh_sb[:, m], el,
                                    start=False, stop=(m >= NWL),
                                    pairs=EL_PAIRS[m])
                        if m < NWL:
                            mm_quad(pm, wl_sb[:, m], eh,
                                    start=False, stop=True)
                    if phase == "h":
                        continue

                    # chunk m is closed: tanh, deferred exp, v-dot
                    ci = len(closed)
                    closed.append(m)
                    tt = ttp.tile([P, NF], BF16, tag="tt")
                    nc.scalar.activation(
                        tt[:],
                        pms[m][:],
                        mybir.ActivationFunctionType.Tanh,
                        bias=dec_sb[:, m, b : b + 1],
                        scale=1.0 / ALPHA,
                    )
                    tts[m] = tt
                    if ci == 0 and pending_exp is not None:
                        # previous block's exp rides the ACT queue AFTER this
                        # block's first tanh, so tanh never waits behind it
                        p_ex, p_so, p_src, p_b = pending_exp
                        nc.scalar.activation(
                            p_ex[:, p_so : p_so + NF],
                            p_src,
                            mybir.ActivationFunctionType.Exp,
                        )
                        # one out-DMA per completed batch row (4 blocks):
                        # the ACT sequencer pays the 632ns HWDGE gen only
                        # once per batch, and on the ACT queue the exp
                        # dependency is already satisfied by queue order
                        if p_so == S - NF:
                            nc.scalar.dma_start(
                                out_d.ap()[p_b : p_b + 1, :], p_ex[:]
                            )
                        elif p_b == BC - 1 and p_so == S - 2 * NF:
                            # last batch: stream out the first 3 slices on
                            # the (idle) sync queue during the final block,
                            # leaving only [1,NF] for the tail DMA
                            nc.sync.dma_start(
                                out_d.ap()[p_b : p_b + 1, 0 : S - NF],
                                p_ex[:, 0 : S - NF],
                            )
                        pending_exp = None
                    if last:
                        # PE v-dot (bf16), deferred two chunks so it rides
                        # behind later chunks' matmuls instead of stalling
                        # the PE on the tanh chain
                        if ci >= 2:
                            pm2 = closed[ci - 2]
                            nc.tensor.matmul(
                                pv[:],
                                vb_sb[:, pm2 : pm2 + 1],
                                tts[pm2][:],
                                start=(ci == 2),
                                stop=False,
                            )
                        if ci == MC - 1:
                            for mj in (closed[ci - 1], m):
                                nc.tensor.matmul(
                                    pv[:],
                                    vb_sb[:, mj : mj + 1],
                                    tts[mj][:],
                                    start=False,
                                    stop=(mj == m),
                                )
                    elif ci == 0:
                        # v-dot accumulation on DVE: vt += tt * v[:,m]
                        nc.vector.tensor_scalar_mul(
                            vt[:], tt[:], v_sb[:, m : m + 1]
                        )
                    else:
                        nc.vector.scalar_tensor_tensor(
                            vt[:],
                            tt[:],
                            v_sb[:, m : m + 1],
                            vt[:],
                            mybir.AluOpType.mult,
                            mybir.AluOpType.add,
                        )

                if last:
                    # tail: emit exp + DMA immediately
                    nc.scalar.activation(
                        ex[:, so : so + NF],
                        pv[0:1, :],
                        mybir.ActivationFunctionType.Exp,
                    )
                    nc.sync.dma_start(
                        out_d.ap()[b : b + 1, S - NF :], ex[:, S - NF :]
                    )
                else:
                    sc = scp.tile([P, NF], F32, tag="sc", name="sc")
                    nc.gpsimd.partition_all_reduce(
                        sc[:], vt[:], P, bass_isa.ReduceOp.add
                    )
                    pending_exp = (ex, so, sc[0:1, :], b)

    nc.compile()
    return nc


def _get_nc():
    if "nc" not in _CACHE:
        _CACHE["nc"] = build()
    return _CACHE["nc"]


def prep_in_maps(decoder_hidden, encoder_outputs, coverage, W_enc, W_dec, b_dec, w_cov, v):
    decoder_hidden = np.asarray(decoder_hidden, dtype=np.float32)
    encoder_outputs = np.asarray(encoder_outputs, dtype=np.float32)
    coverage = np.asarray(coverage, dtype=np.float32)
    W_enc = np.asarray(W_enc, dtype=np.float32)
    W_dec = np.asarray(W_dec, dtype=np.float32)
    b_dec = np.asarray(b_dec, dtype=np.float32)
    w_cov = np.asarray(w_cov, dtype=np.float32)
    v = np.asarray(v, dtype=np.float32)

    # host-side tiny matmul: dec_feature [B, H]
    dec_feature = decoder_hidden[:, 0, :] @ W_dec.T + b_dec

    # Channel permutation by |v| descending: the attn error from dropped
    # correction passes scales with v_h^2, so corrections go to the top
    # chunks only.
    perm = np.argsort(-np.abs(v))
    vp = v[perm]
    Wp = W_enc[perm, :]
    wcovp = w_cov[perm]
    decp = dec_feature[:, perm]

    # W.T scaled by 32 (exact power of 2), split hi/lo into e4m3 with error
    # feedback. The x32 keeps Wl out of fp8 subnormal underflow.
    w32 = np.ascontiguousarray(Wp.T) * np.float32(ALPHA)      # [H(in), H(out)]
    wh8 = w32.astype(E4NP)
    wl8 = (w32 - wh8.astype(np.float32)).astype(E4NP)

    # Coverage fold: u s.t. u @ w32 ~ wcov*ALPHA via truncated SVD, so the
    # rank-1 cov term rides inside e and needs no device work at all.
    U, sv, Vt = np.linalg.svd(w32.astype(np.float64))
    keep = sv >= FOLD_EPS * sv[0]
    coef = Vt @ (wcovp.astype(np.float64) * ALPHA)
    u_fold = (U[:, keep] @ (coef[keep] / sv[keep])).astype(np.float32)

    def w_rearrange(w8, mc):
        # [H, mc*P] = [(k p), (m c)] -> [p, (m k c)] so per-m-chunk DMAs are
        # contiguous per partition
        return np.ascontiguousarray(
            w8.reshape(KC, P, mc, P).transpose(1, 2, 0, 3).reshape(P, mc * KC * P)
        )

    wh8 = w_rearrange(wh8, MC)
    wl8 = w_rearrange(wl8[:, : NWL * P], NWL)
    v_r = vp.reshape(MC, P).T                                 # [P, MC] f32
    vb_r = np.ascontiguousarray(v_r.astype(ml_dtypes.bfloat16))

    in_maps = []
    for c in range(NCORES):
        bs = slice(c * BC, (c + 1) * BC)
        e2 = encoder_outputs[bs] + coverage[bs][..., None] * u_fold
        encT = np.ascontiguousarray(e2.reshape(R, H).T)       # [H, R]
        eh8 = encT.astype(E4NP)
        el8 = (encT - eh8.astype(np.float32)).astype(E4NP)
        dec = decp[bs].T.reshape(MC, P, BC).transpose(1, 0, 2)  # [P, MC, BC]
        cst = np.ascontiguousarray(
            np.concatenate([v_r, dec.reshape(P, MC * BC)], axis=1).astype(
                np.float32
            )
        )
        in_maps.append(
            {
                "eh": eh8,
                "el": el8,
                "wh": wh8,
                "wl": wl8,
                "cst": cst,
                "vb": vb_r,
            }
        )
    return in_maps


def kernel(decoder_hidden, encoder_outputs, coverage, W_enc, W_dec, b_dec, w_cov, v):
    nc = _get_nc()
    in_maps = prep_in_maps(
        decoder_hidden, encoder_outputs, coverage, W_enc, W_dec, b_dec, w_cov, v
    )
    res = run_bass_kernel_spmd(nc, in_maps, core_ids=list(range(NCORES)))
    out = np.concatenate([r["attn"] for r in res.results], axis=0)  # [B, S] exp
    out = out / out.sum(axis=-1, keepdims=True)                     # normalize
    return out[:, None, :].astype(np.float32)                       # [B, 1, S]
